# revision 7
# baseline (speedup 1.0000x reference)
"""Trainium2 Bass kernel for a 3-layer GIN encoder (gnn_message_passing).

Reference computation (per layer l):
    agg_i = sum_{j -> i} z_j          (scatter-add over edges)
    h     = z + agg                   (GIN eps=0, folded in as self-edges)
    z     = relu(relu(h @ w1 + b1) @ w2 + b2)

Distribution strategy (8 NeuronCores, SPMD single program):
  * Nodes are block-sharded: core c owns rows [c*NPC, (c+1)*NPC).
  * Edges are partitioned by destination core; the aggregation is local.
  * Aggregation runs on TensorE as a dense matmul with the local adjacency
    count matrix:  h.T = z.T @ Aloc.T  where Aloc[dst, src] counts edges
    (incl. one self-edge per node).  Aloc.T (bf16, exact small ints) is
    precomputed on the host and streamed from HBM in K-chunks.
  * The layer-boundary halo exchange (AllGather of each core's activation
    rows) is split into 3 pieces aligned with the MLP output groups and
    PIPELINED: piece g is gathered as soon as MLP group g stores its rows,
    and the next layer's aggregation consumes piece g's K-chunks as soon
    as that piece lands.  The source-slot order of A.T is permuted on the
    host to match (piece-major, then core, then row).
  * A dummy 16-byte AllGather issued first absorbs the collectives' global
    start barrier into the initial weight/adjacency load phase.
  * MLP runs in plain bf16 (weights + activations; fp32 PSUM accumulate),
    which keeps rel err ~4e-3 vs the 2e-2 budget.
"""

import os
import sys

sys.path.insert(0, "/opt/trn_rl_repo")

import numpy as np
import ml_dtypes

BF16 = ml_dtypes.bfloat16
P = 128
NCORES = 8
NPC = 1250            # nodes per core (N=10000)
MT = 10               # dst M-tiles per core
NSLOT = NPC           # dst slots per core (trimmed, no pad)

# source-slot pieces per core: (row_off, valid_rows, padded_rows)
PIECES = [(0, 512, 512), (512, 512, 512), (1024, 226, 256)]
BASES = [0, NCORES * 512, NCORES * 1024]      # global slot base per piece
SPC = sum(p[2] for p in PIECES)               # padded slots per core (1280)
KC = NCORES * SPC // P                        # src K-chunks (80)

# adjacency K-chunks fetched per DMA (batching amortizes HWDGE issue cost)
ABATCH = 4
# adjacency K-chunks kept resident in SBUF across layers
ACACHE = 20
# cached chunks issued before the first streamed batch of each layer
CFIRST = 20

_BUILD_CACHE: dict = {}


# --------------------------------------------------------------------------
# host-side preprocessing
# --------------------------------------------------------------------------

def _config(inputs):
    x = inputs["x"]
    N, DIN = int(x.shape[0]), int(x.shape[1])
    L = 0
    while f"w1_{L}" in inputs:
        L += 1
    DH = int(inputs["w1_0"].shape[1])
    assert N == NCORES * NPC
    assert DIN % P == 0 and DH % P == 0
    return dict(N=N, DIN=DIN, DH=DH, L=L)


def _src_slot(node):
    """Global src node id -> permuted K-dim slot (piece-major layout)."""
    c = node // NPC
    r = node - c * NPC
    slot = np.empty_like(node)
    for (off, valid, pad), base in zip(PIECES, BASES):
        m = (r >= off) & (r < off + valid)
        slot[m] = base + c[m] * pad + (r[m] - off)
    return slot


def _prep_at(edge_index, N):
    """Dense transposed local adjacency per core, src-permuted.

    Returns per-core arrays with at[k, p, s] = #edges
    (src slot = k*128+p) -> (dst = c*NPC + s), plus the identity
    (self-edge).  Split into an SBUF-cached head and batched streams.
    """
    src = np.asarray(edge_index[0], dtype=np.int64)
    dst = np.asarray(edge_index[1], dtype=np.int64)
    self_ix = np.arange(N, dtype=np.int64)
    allsrc = np.concatenate([src, self_ix])
    alldst = np.concatenate([dst, self_ix])

    slot = _src_slot(allsrc)
    dcore = alldst // NPC
    ds = alldst - dcore * NPC

    at = np.zeros((KC * P, NCORES * NSLOT), np.float32)
    np.add.at(at, (slot, dcore * NSLOT + ds), 1.0)
    at_bf = at.astype(BF16)
    at_u8 = at.astype(np.uint8)

    CA = min(ACACHE, KC)
    NSB = -(-(KC - CA) // ABATCH)          # streamed batches (padded)
    CAB = -(-CA // ABATCH)                 # cache-load batches

    def batched(arr, lo, hi, nb):
        # [chunks, P, NSLOT] -> [nb, P, ABATCH*NSLOT], batch-contiguous per
        # partition so each stream DMA moves ABATCH*NSLOT*esz contiguous
        # bytes per partition
        sl = arr[lo:hi]
        pad = nb * ABATCH - (hi - lo)
        if pad:
            sl = np.concatenate(
                [sl, np.zeros((pad,) + sl.shape[1:], sl.dtype)], axis=0)
        return np.ascontiguousarray(
            sl.reshape(nb, ABATCH, P, NSLOT)
              .transpose(0, 2, 1, 3)
              .reshape(nb, P, ABATCH * NSLOT))

    aca, ats, at8s = [], [], []
    for c in range(NCORES):
        sl_bf = at_bf[:, c * NSLOT:(c + 1) * NSLOT].reshape(KC, P, NSLOT)
        sl_u8 = at_u8[:, c * NSLOT:(c + 1) * NSLOT].reshape(KC, P, NSLOT)
        aca.append(batched(sl_bf, 0, CA, CAB))
        ats.append(batched(sl_bf, CA, KC, NSB))
        at8s.append(batched(sl_u8, CA, KC, NSB))
    return aca, ats, at8s


# --------------------------------------------------------------------------
# bass program
# --------------------------------------------------------------------------

def _build(N, DIN, DH, L):
    from concourse import bacc, mybir, tile

    f32 = mybir.dt.float32
    bf = mybir.dt.bfloat16
    RELU = mybir.ActivationFunctionType.Relu

    NKT2 = DH // P  # K/M tiles of the hidden dim (2)
    GROUPS = [(g0, min(4, MT - g0)) for g0 in range(0, MT, 4)]
    # slot groups for the aggregation matmul free dim (<=512 per PSUM bank)
    NG = [(n0, min(512, NSLOT - n0)) for n0 in range(0, NSLOT, 512)]

    nc = bacc.Bacc(num_devices=NCORES)

    xin = nc.dram_tensor("x_bf", [N, DIN], bf, kind="ExternalInput")
    CA = min(ACACHE, KC)
    NSB = -(-(KC - CA) // ABATCH)
    CAB = -(-CA // ABATCH)
    acain = nc.dram_tensor("aca", [CAB, P, ABATCH * NSLOT], bf,
                           kind="ExternalInput")
    atsin = nc.dram_tensor("ats", [NSB, P, ABATCH * NSLOT], bf,
                           kind="ExternalInput")
    at8in = nc.dram_tensor("at8s", [NSB, P, ABATCH * NSLOT], mybir.dt.uint8,
                           kind="ExternalInput")
    identbin = nc.dram_tensor("identb", [P, P], bf, kind="ExternalInput")
    dumin = nc.dram_tensor("dummyin", [8, 2], bf, kind="ExternalInput")
    win = {}
    for l in range(L):
        din = DIN if l == 0 else DH
        for nm, shp in [("w1h", [din, DH]), ("w2h", [DH, DH])]:
            win[(nm, l)] = nc.dram_tensor(f"{nm}_{l}", shp, bf, kind="ExternalInput")
        for nm in ("b1", "b2"):
            win[(nm, l)] = nc.dram_tensor(f"{nm}_{l}", [DH, 1], f32, kind="ExternalInput")
    zout = nc.dram_tensor("zout", [NPC, DH], bf, kind="ExternalOutput")

    with tile.TileContext(nc) as tc:
        with tc.tile_pool(name="const", bufs=1) as cp, \
             tc.tile_pool(name="atpool", bufs=5) as atp, \
             tc.tile_pool(name="a8pool", bufs=4) as a8p, \
             tc.tile_pool(name="zsbpool", bufs=1) as zsp, \
             tc.tile_pool(name="hpool", bufs=1) as hp, \
             tc.tile_pool(name="spool", bufs=2) as sp, \
             tc.tile_pool(name="zpool", bufs=1) as zp, \
             tc.tile_pool(name="zrpool", bufs=3) as zrp, \
             tc.tile_pool(name="hpsum", bufs=1, space="PSUM") as hpsum, \
             tc.tile_pool(name="mlppsum", bufs=2, space="PSUM") as mlppool, \
             tc.tile_pool(name="drampool", bufs=1, space="DRAM") as dp:

            # dummy collective first: absorbs the global start barrier into
            # the initial load phase instead of blocking layer-0 compute
            # (collectives can't read IO tensors, so bounce via internal DRAM)
            dummy_i = dp.tile([8, 2], bf, name="dummy_i")
            dummy_o = dp.tile([NCORES * 8, 2], bf, name="dummy_o",
                              addr_space="Shared")
            nc.gpsimd.dma_start(out=dummy_i[:, :], in_=dumin[:, :])
            nc.gpsimd.collective_compute(
                "AllGather",
                mybir.AluOpType.bypass,
                replica_groups=[list(range(NCORES))],
                ins=[dummy_i[:, :].opt()],
                outs=[dummy_o[:, :].opt()],
            )

            # ---------------- resident constants ----------------
            identb_t = cp.tile([P, P], bf, name="identb_t")
            nc.gpsimd.dma_start(out=identb_t[:], in_=identbin[:, :])

            # resident head of the adjacency (reused by all layers)
            acache = cp.tile([P, CA * NSLOT], bf, name="acache")
            for b in range(CAB):
                nc.gpsimd.dma_start(
                    out=acache[:, b * ABATCH * NSLOT:(b + 1) * ABATCH * NSLOT],
                    in_=acain[b, :, :])

            wt = {}
            for l in range(L):
                din = DIN if l == 0 else DH
                nkt = din // P
                for nm, nk in (("w1h", nkt), ("w2h", NKT2)):
                    t = cp.tile([P, nk * DH], bf, name=f"{nm}{l}_t")
                    for kt in range(nk):
                        nc.gpsimd.dma_start(
                            out=t[:, kt * DH:(kt + 1) * DH],
                            in_=win[(nm, l)][kt * P:(kt + 1) * P, :])
                    wt[(nm, l)] = t
                for nm in ("b1", "b2"):
                    t = cp.tile([P, NKT2], f32, name=f"{nm}{l}_t")
                    for mo in range(NKT2):
                        nc.gpsimd.dma_start(
                            out=t[:, mo:mo + 1],
                            in_=win[(nm, l)][mo * P:(mo + 1) * P, :])
                    wt[(nm, l)] = t

            # layer-boundary activation tables (per piece, AllGather outputs)
            zloc = [dp.tile([NPC, DH], bf, name=f"zloc{l}") for l in range(L - 1)]
            zfp = [[dp.tile([NCORES * PIECES[g][1], DH], bf,
                            name=f"zf{l}_{g}", addr_space="Shared")
                    for g in range(len(PIECES))]
                   for l in range(L - 1)]

            # ---------------- layers ----------------
            for l in range(L):
                din = DIN if l == 0 else DH
                nkt = din // P
                last = (l == L - 1)

                # activation table -> SBUF, chunked [128, KC*din]:
                # zsb[p, k*din+f] = z[slot k*128+p, f], piece-by-piece so the
                # K-loop matmuls can start as soon as each AllGather lands
                zsb = zsp.tile([P, KC * din], bf, name=f"zsb_{l}", tag="zsb")
                # zero the pad slots (piece 2 rows 226..255 per core) so the
                # matmuls never see NaN-pattern garbage
                for c in range(NCORES):
                    cb = (BASES[2] + c * PIECES[2][2]) // P
                    nc.vector.memset(
                        zsb[:, (cb + 1) * din:(cb + 2) * din], 0.0)
                for g, (off, valid, pad) in enumerate(PIECES):
                    base = BASES[g]
                    if l == 0:
                        for c in range(NCORES):
                            rows0 = c * NPC + off
                            cb = (base + c * pad) // P
                            kfull = valid // P
                            if kfull:
                                nc.scalar.dma_start(
                                    out=zsb[:, cb * din:(cb + kfull) * din]
                                        .rearrange("p (k f) -> p k f", f=din),
                                    in_=xin[rows0:rows0 + kfull * P, :]
                                        .rearrange("(k p) f -> p k f", p=P))
                            rem = valid - kfull * P
                            if rem:
                                nc.scalar.dma_start(
                                    out=zsb[:rem, (cb + kfull) * din:
                                            (cb + kfull + 1) * din],
                                    in_=xin[rows0 + kfull * P:rows0 + valid, :])
                    else:
                        t = zfp[l - 1][g]
                        if valid == pad:
                            k = NCORES * valid // P
                            cb = base // P
                            for k0 in range(0, k, 8):
                                k1 = min(k, k0 + 8)
                                nc.scalar.dma_start(
                                    out=zsb[:, (cb + k0) * din:(cb + k1) * din]
                                        .rearrange("p (k f) -> p k f", f=din),
                                    in_=t[k0 * P:k1 * P, :]
                                        .rearrange("(k p) f -> p k f", p=P))
                        else:
                            for c in range(NCORES):
                                rows0 = c * valid
                                cb = (base + c * pad) // P
                                kfull = valid // P
                                if kfull:
                                    nc.scalar.dma_start(
                                        out=zsb[:, cb * din:(cb + kfull) * din]
                                            .rearrange("p (k f) -> p k f", f=din),
                                        in_=t[rows0:rows0 + kfull * P, :]
                                            .rearrange("(k p) f -> p k f", p=P))
                                rem = valid - kfull * P
                                if rem:
                                    nc.scalar.dma_start(
                                        out=zsb[:rem, (cb + kfull) * din:
                                                (cb + kfull + 1) * din],
                                        in_=t[rows0 + kfull * P:rows0 + valid, :])

                # --- aggregation: h.T = z.T @ Aloc.T  (PSUM-accumulated)
                hps = [hpsum.tile([P, len(NG) * 512], f32,
                                  name=f"hps{mf}_{l}", tag=f"hps{mf}")
                       for mf in range(nkt)]

                def agg_mms(k, rhs_tile, rhs_off, first, final):
                    for mf in range(nkt):
                        for gi, (n0, nn) in enumerate(NG):
                            nc.tensor.matmul(
                                out=hps[mf][:, gi * 512: gi * 512 + nn],
                                lhsT=zsb[:, k * din + mf * P: k * din + (mf + 1) * P],
                                rhs=rhs_tile[:, rhs_off + n0: rhs_off + n0 + nn],
                                start=first,
                                stop=final,
                            )

                # Interleave SBUF-cached chunks between streamed batches so
                # TensorE never stalls on the A.T stream.
                cached = list(range(CA))
                head, rest = cached[:CFIRST], cached[CFIRST:]
                seq = [("C", k) for k in head]
                nb = max(1, NSB)
                per = [len(rest) * (bi + 1) // nb for bi in range(nb)]
                ci = 0
                for bi in range(NSB):
                    seq.append(("S", bi))
                    while ci < per[bi]:
                        seq.append(("C", rest[ci]))
                        ci += 1
                while ci < len(rest):
                    seq.append(("C", rest[ci]))
                    ci += 1

                nchunks = KC
                done = 0
                for kind, payload in seq:
                    if kind == "S":
                        b = payload
                        ks = [CA + b * ABATCH + j for j in range(ABATCH)
                              if CA + b * ABATCH + j < KC]
                        at_t = atp.tile([P, ABATCH * NSLOT], bf,
                                        name=f"at_{l}_{b}", tag="at")
                        if l == 0:
                            # layer 0 is DMA-bound: stream uint8, cast to
                            # bf16 on the otherwise-idle DVE/ACT engines
                            at8_t = a8p.tile([P, ABATCH * NSLOT],
                                             mybir.dt.uint8,
                                             name=f"at8_{l}_{b}", tag="at8")
                            nc.sync.dma_start(out=at8_t[:], in_=at8in[b, :, :])
                            cut = ABATCH * NSLOT * 5 // 8
                            nc.vector.tensor_copy(
                                out=at_t[:, :cut], in_=at8_t[:, :cut])
                            nc.gpsimd.tensor_copy(
                                out=at_t[:, cut:], in_=at8_t[:, cut:])
                        else:
                            nc.sync.dma_start(out=at_t[:], in_=atsin[b, :, :])
                        for k in ks:
                            agg_mms(k, at_t, (k - CA - b * ABATCH) * NSLOT,
                                    done == 0, done == nchunks - 1)
                            done += 1
                    else:
                        k = payload
                        agg_mms(k, acache, k * NSLOT,
                                done == 0, done == nchunks - 1)
                        done += 1

                # --- h.T -> bf16 (plain, no hi/lo split)
                hhi = [hp.tile([P, MT * P], bf, name=f"hhi{mf}_{l}", tag=f"hhi{mf}")
                       for mf in range(nkt)]
                for mf in range(nkt):
                    for gi, (n0, nn) in enumerate(NG):
                        nc.vector.tensor_copy(
                            out=hhi[mf][:, n0:n0 + nn],
                            in_=hps[mf][:, gi * 512: gi * 512 + nn])

                # --- MLP over groups of 4 M-tiles (512-row free dim)
                zT = [zp.tile([P, MT * P], bf, name=f"zT{mo}_{l}",
                              tag=f"zT{mo}")
                      for mo in range(NKT2)]
                for gi, (g0, gm) in enumerate(GROUPS):
                    rows = gm * P
                    r0 = g0 * P
                    s1h = []
                    for mo in range(NKT2):
                        p1 = mlppool.tile([P, 512], f32,
                                          name=f"p1_{l}_{g0}_{mo}", tag="mlp")
                        for kt in range(nkt):
                            nc.tensor.matmul(
                                out=p1[:, :rows],
                                lhsT=wt[("w1h", l)][:, kt * DH + mo * P: kt * DH + (mo + 1) * P],
                                rhs=hhi[kt][:, r0:r0 + rows],
                                start=(kt == 0), stop=(kt == nkt - 1))
                        sh = sp.tile([P, 512], bf, name=f"s1h_{l}_{g0}_{mo}", tag=f"s1h{mo}")
                        nc.scalar.activation(
                            out=sh[:, :rows], in_=p1[:, :rows], func=RELU,
                            bias=wt[("b1", l)][:, mo:mo + 1])
                        s1h.append(sh)
                    for mo in range(NKT2):
                        p2 = mlppool.tile([P, 512], f32,
                                          name=f"p2_{l}_{g0}_{mo}", tag="mlp")
                        for kt in range(NKT2):
                            nc.tensor.matmul(
                                out=p2[:, :rows],
                                lhsT=wt[("w2h", l)][:, kt * DH + mo * P: kt * DH + (mo + 1) * P],
                                rhs=s1h[kt][:, :rows],
                                start=(kt == 0), stop=(kt == NKT2 - 1))
                        nc.scalar.activation(
                            out=zT[mo][:, r0:r0 + rows], in_=p2[:, :rows], func=RELU,
                            bias=wt[("b2", l)][:, mo:mo + 1])

                    # transpose this group's M-tiles back to row-major + store
                    for m in range(g0, g0 + gm):
                        rows_m = min(P, NPC - m * P)
                        tp = mlppool.tile([P, NKT2 * P], bf,
                                          name=f"tp_{l}_{m}", tag="mlp")
                        for mo in range(NKT2):
                            nc.tensor.transpose(
                                out=tp[:, mo * P:(mo + 1) * P],
                                in_=zT[mo][:, m * P:(m + 1) * P],
                                identity=identb_t[:])
                        zr = zrp.tile([P, NKT2 * P], bf,
                                      name=f"zr_{l}_{m}", tag="zr")
                        nc.vector.tensor_copy(out=zr[:], in_=tp[:])
                        dst = zout if last else zloc[l]
                        nc.sync.dma_start(
                            out=dst[m * P: m * P + rows_m, :],
                            in_=zr[:rows_m, :])

                    # halo exchange for this group's rows, overlapped with
                    # the rest of the MLP and the next layer's aggregation
                    if not last:
                        off, valid, pad = PIECES[gi]
                        nc.gpsimd.collective_compute(
                            "AllGather",
                            mybir.AluOpType.bypass,
                            replica_groups=[list(range(NCORES))],
                            ins=[zloc[l][off:off + valid, :].opt()],
                            outs=[zfp[l][gi][:, :].opt()],
                        )

    # populates extended-inst ISA bytes + inserts GPSIMD library loads
    nc.compile()
    return nc


# --------------------------------------------------------------------------
# entry point
# --------------------------------------------------------------------------

def _make_in_maps(inputs, cfg, aca, ats, at8s):
    DH, L = cfg["DH"], cfg["L"]
    x_bf = np.ascontiguousarray(np.asarray(inputs["x"], dtype=np.float32)).astype(BF16)
    identb = np.eye(P, dtype=np.float32).astype(BF16)

    shared = {"x_bf": x_bf, "identb": identb,
              "dummyin": np.zeros((8, 2), BF16)}
    for l in range(L):
        w1 = np.asarray(inputs[f"w1_{l}"], dtype=np.float32)
        w2 = np.asarray(inputs[f"w2_{l}"], dtype=np.float32)
        shared[f"w1h_{l}"] = w1.astype(BF16)
        shared[f"w2h_{l}"] = w2.astype(BF16)
        shared[f"b1_{l}"] = np.asarray(
            inputs[f"b1_{l}"], dtype=np.float32).reshape(DH, 1)
        shared[f"b2_{l}"] = np.asarray(
            inputs[f"b2_{l}"], dtype=np.float32).reshape(DH, 1)

    in_maps = []
    for c in range(NCORES):
        m = dict(shared)
        m["aca"] = aca[c]
        m["ats"] = ats[c]
        m["at8s"] = at8s[c]
        in_maps.append(m)
    return in_maps


def get_program(inputs):
    """Build (or fetch cached) the bass program + per-core input maps."""
    cfg = _config(inputs)
    aca, ats, at8s = _prep_at(inputs["edge_index"], cfg["N"])
    key = (cfg["N"], cfg["DIN"], cfg["DH"], cfg["L"])
    if key not in _BUILD_CACHE:
        _BUILD_CACHE[key] = _build(cfg["N"], cfg["DIN"], cfg["DH"], cfg["L"])
    nc = _BUILD_CACHE[key]
    in_maps = _make_in_maps(inputs, cfg, aca, ats, at8s)
    return nc, in_maps, cfg


def kernel(**inputs):
    nc, in_maps, cfg = get_program(inputs)

    if os.environ.get("KERNEL_USE_SIM"):
        from concourse.bass_interp import MultiCoreSim
        sim = MultiCoreSim(nc, num_cores=NCORES)
        cores = list(sim.cores.values())
        for cid, cs in enumerate(cores):
            for name, val in in_maps[cid].items():
                cs.tensor(name)[:] = val
        sim.simulate(check_with_hw=False)
        parts = [np.asarray(cs.tensor("zout")) for cs in cores]
    else:
        from concourse import bass_utils
        res = bass_utils.run_bass_kernel_spmd(
            nc, in_maps, core_ids=list(range(NCORES)),
            trace=bool(os.environ.get("KERNEL_TRACE")),
        )
        kernel.last_results = res
        parts = [res.results[c]["zout"] for c in range(NCORES)]

    out = np.concatenate(parts, axis=0).astype(np.float32)
    return out


# revision 8
# speedup vs baseline: 1.1973x; 1.1973x over previous
"""Trainium2 Bass kernel for a 3-layer GIN encoder (gnn_message_passing).

Reference computation (per layer l):
    agg_i = sum_{j -> i} z_j          (scatter-add over edges)
    h     = z + agg                   (GIN eps=0, folded in as self-edges)
    z     = relu(relu(h @ w1 + b1) @ w2 + b2)

Distribution strategy (8 NeuronCores, SPMD single program):
  * Nodes are block-sharded: core c owns rows [c*NPC, (c+1)*NPC).
  * Edges are partitioned by destination core; the aggregation is local.
  * Aggregation runs on TensorE as a dense matmul with the local adjacency
    count matrix:  h.T = z.T @ Aloc.T  where Aloc[dst, src] counts edges
    (incl. one self-edge per node).  Aloc.T (bf16, exact small ints) is
    precomputed on the host and streamed from HBM in K-chunks.
  * The layer-boundary halo exchange (AllGather of each core's activation
    rows) is split into 3 pieces aligned with the MLP output groups and
    PIPELINED: piece g is gathered as soon as MLP group g stores its rows,
    and the next layer's aggregation consumes piece g's K-chunks as soon
    as that piece lands.  The source-slot order of A.T is permuted on the
    host to match (piece-major, then core, then row).
  * A dummy 16-byte AllGather issued first absorbs the collectives' global
    start barrier into the initial weight/adjacency load phase.
  * MLP runs in plain bf16 (weights + activations; fp32 PSUM accumulate),
    which keeps rel err ~4e-3 vs the 2e-2 budget.
"""

import os
import sys

sys.path.insert(0, "/opt/trn_rl_repo")

import numpy as np
import ml_dtypes

BF16 = ml_dtypes.bfloat16
P = 128
NCORES = 8
NPC = 1250            # nodes per core (N=10000)
MT = 10               # dst M-tiles per core
NSLOT = NPC           # dst slots per core (trimmed, no pad)

# source-slot pieces per core: (row_off, valid_rows, padded_rows)
PIECES = [(0, 512, 512), (512, 512, 512), (1024, 226, 256)]
BASES = [0, NCORES * 512, NCORES * 1024]      # global slot base per piece
SPC = sum(p[2] for p in PIECES)               # padded slots per core (1280)
KC = NCORES * SPC // P                        # src K-chunks (80)

# adjacency K-chunks fetched per DMA (batching amortizes HWDGE issue cost)
ABATCH = 4
# adjacency K-chunks kept resident in SBUF across layers
ACACHE = 20
# cached chunks issued before the first streamed batch of each layer
CFIRST = 20

_BUILD_CACHE: dict = {}


# --------------------------------------------------------------------------
# host-side preprocessing
# --------------------------------------------------------------------------

def _config(inputs):
    x = inputs["x"]
    N, DIN = int(x.shape[0]), int(x.shape[1])
    L = 0
    while f"w1_{L}" in inputs:
        L += 1
    DH = int(inputs["w1_0"].shape[1])
    assert N == NCORES * NPC
    assert DIN % P == 0 and DH % P == 0
    return dict(N=N, DIN=DIN, DH=DH, L=L)


def _src_slot(node):
    """Global src node id -> permuted K-dim slot (piece-major layout)."""
    c = node // NPC
    r = node - c * NPC
    slot = np.empty_like(node)
    for (off, valid, pad), base in zip(PIECES, BASES):
        m = (r >= off) & (r < off + valid)
        slot[m] = base + c[m] * pad + (r[m] - off)
    return slot


def _prep_at(edge_index, N):
    """Dense transposed local adjacency per core, src-permuted.

    Returns per-core arrays with at[k, p, s] = #edges
    (src slot = k*128+p) -> (dst = c*NPC + s), plus the identity
    (self-edge).  Split into an SBUF-cached head and batched streams.
    """
    src = np.asarray(edge_index[0], dtype=np.int64)
    dst = np.asarray(edge_index[1], dtype=np.int64)
    self_ix = np.arange(N, dtype=np.int64)
    allsrc = np.concatenate([src, self_ix])
    alldst = np.concatenate([dst, self_ix])

    slot = _src_slot(allsrc)
    dcore = alldst // NPC
    ds = alldst - dcore * NPC

    at = np.zeros((KC * P, NCORES * NSLOT), np.float32)
    np.add.at(at, (slot, dcore * NSLOT + ds), 1.0)
    at_bf = at.astype(BF16)
    at_u8 = at.astype(np.uint8)

    CA = min(ACACHE, KC)
    NSB = -(-(KC - CA) // ABATCH)          # streamed batches (padded)
    CAB = -(-CA // ABATCH)                 # cache-load batches

    def batched(arr, lo, hi, nb):
        # [chunks, P, NSLOT] -> [nb, P, ABATCH*NSLOT], batch-contiguous per
        # partition so each stream DMA moves ABATCH*NSLOT*esz contiguous
        # bytes per partition
        sl = arr[lo:hi]
        pad = nb * ABATCH - (hi - lo)
        if pad:
            sl = np.concatenate(
                [sl, np.zeros((pad,) + sl.shape[1:], sl.dtype)], axis=0)
        return np.ascontiguousarray(
            sl.reshape(nb, ABATCH, P, NSLOT)
              .transpose(0, 2, 1, 3)
              .reshape(nb, P, ABATCH * NSLOT))

    aca, ats, at8s = [], [], []
    for c in range(NCORES):
        sl_bf = at_bf[:, c * NSLOT:(c + 1) * NSLOT].reshape(KC, P, NSLOT)
        sl_u8 = at_u8[:, c * NSLOT:(c + 1) * NSLOT].reshape(KC, P, NSLOT)
        aca.append(batched(sl_bf, 0, CA, CAB))
        ats.append(batched(sl_bf, CA, KC, NSB))
        at8s.append(batched(sl_u8, CA, KC, NSB))
    return aca, ats, at8s


# --------------------------------------------------------------------------
# bass program
# --------------------------------------------------------------------------

def _build(N, DIN, DH, L):
    from concourse import bacc, mybir, tile

    f32 = mybir.dt.float32
    bf = mybir.dt.bfloat16
    RELU = mybir.ActivationFunctionType.Relu

    NKT2 = DH // P  # K/M tiles of the hidden dim (2)
    GROUPS = [(g0, min(4, MT - g0)) for g0 in range(0, MT, 4)]
    # slot groups for the aggregation matmul free dim (<=512 per PSUM bank)
    NG = [(n0, min(512, NSLOT - n0)) for n0 in range(0, NSLOT, 512)]

    nc = bacc.Bacc(num_devices=NCORES)

    xin = nc.dram_tensor("x_bf", [N, DIN], bf, kind="ExternalInput")
    CA = min(ACACHE, KC)
    NSB = -(-(KC - CA) // ABATCH)
    CAB = -(-CA // ABATCH)
    acain = nc.dram_tensor("aca", [CAB, P, ABATCH * NSLOT], bf,
                           kind="ExternalInput")
    atsin = nc.dram_tensor("ats", [NSB, P, ABATCH * NSLOT], bf,
                           kind="ExternalInput")
    at8in = nc.dram_tensor("at8s", [NSB, P, ABATCH * NSLOT], mybir.dt.uint8,
                           kind="ExternalInput")
    identbin = nc.dram_tensor("identb", [P, P], bf, kind="ExternalInput")
    dumin = nc.dram_tensor("dummyin", [8, 2], bf, kind="ExternalInput")
    win = {}
    for l in range(L):
        din = DIN if l == 0 else DH
        for nm, shp in [("w1h", [din, DH]), ("w2h", [DH, DH])]:
            win[(nm, l)] = nc.dram_tensor(f"{nm}_{l}", shp, bf, kind="ExternalInput")
        for nm in ("b1", "b2"):
            win[(nm, l)] = nc.dram_tensor(f"{nm}_{l}", [DH, 1], f32, kind="ExternalInput")
    zout = nc.dram_tensor("zout", [NPC, DH], bf, kind="ExternalOutput")

    with tile.TileContext(nc) as tc:
        with tc.tile_pool(name="const", bufs=1) as cp, \
             tc.tile_pool(name="atpool", bufs=5) as atp, \
             tc.tile_pool(name="a8pool", bufs=4) as a8p, \
             tc.tile_pool(name="zsbpool", bufs=1) as zsp, \
             tc.tile_pool(name="hpool", bufs=1) as hp, \
             tc.tile_pool(name="spool", bufs=2) as sp, \
             tc.tile_pool(name="zpool", bufs=1) as zp, \
             tc.tile_pool(name="zrpool", bufs=3) as zrp, \
             tc.tile_pool(name="hpsum", bufs=1, space="PSUM") as hpsum, \
             tc.tile_pool(name="mlppsum", bufs=2, space="PSUM") as mlppool, \
             tc.tile_pool(name="drampool", bufs=1, space="DRAM") as dp:

            # dummy collective first: absorbs the global start barrier into
            # the initial load phase instead of blocking layer-0 compute
            # (collectives can't read IO tensors, so bounce via internal DRAM)
            dummy_i = dp.tile([8, 2], bf, name="dummy_i")
            dummy_o = dp.tile([NCORES * 8, 2], bf, name="dummy_o",
                              addr_space="Shared")
            nc.gpsimd.dma_start(out=dummy_i[:, :], in_=dumin[:, :])
            nc.gpsimd.collective_compute(
                "AllGather",
                mybir.AluOpType.bypass,
                replica_groups=[list(range(NCORES))],
                ins=[dummy_i[:, :].opt()],
                outs=[dummy_o[:, :].opt()],
            )

            # ---------------- resident constants ----------------
            identb_t = cp.tile([P, P], bf, name="identb_t")
            nc.gpsimd.dma_start(out=identb_t[:], in_=identbin[:, :])

            # resident head of the adjacency (reused by all layers)
            acache = cp.tile([P, CA * NSLOT], bf, name="acache")
            for b in range(CAB):
                nc.gpsimd.dma_start(
                    out=acache[:, b * ABATCH * NSLOT:(b + 1) * ABATCH * NSLOT],
                    in_=acain[b, :, :])

            wt = {}
            for l in range(L):
                din = DIN if l == 0 else DH
                nkt = din // P
                for nm, nk in (("w1h", nkt), ("w2h", NKT2)):
                    t = cp.tile([P, nk * DH], bf, name=f"{nm}{l}_t")
                    for kt in range(nk):
                        nc.gpsimd.dma_start(
                            out=t[:, kt * DH:(kt + 1) * DH],
                            in_=win[(nm, l)][kt * P:(kt + 1) * P, :])
                    wt[(nm, l)] = t
                for nm in ("b1", "b2"):
                    t = cp.tile([P, NKT2], f32, name=f"{nm}{l}_t")
                    for mo in range(NKT2):
                        nc.gpsimd.dma_start(
                            out=t[:, mo:mo + 1],
                            in_=win[(nm, l)][mo * P:(mo + 1) * P, :])
                    wt[(nm, l)] = t

            # layer-boundary activation tables (per piece, AllGather outputs)
            zloc = [dp.tile([NPC, DH], bf, name=f"zloc{l}") for l in range(L - 1)]
            zfp = [[dp.tile([NCORES * PIECES[g][1], DH], bf,
                            name=f"zf{l}_{g}", addr_space="Shared")
                    for g in range(len(PIECES))]
                   for l in range(L - 1)]

            # ---------------- layers ----------------
            for l in range(L):
                din = DIN if l == 0 else DH
                nkt = din // P
                last = (l == L - 1)

                # activation table -> SBUF, chunked [128, KC*din]:
                # zsb[p, k*din+f] = z[slot k*128+p, f], piece-by-piece so the
                # K-loop matmuls can start as soon as each AllGather lands
                zsb = zsp.tile([P, KC * din], bf, name=f"zsb_{l}", tag="zsb")
                # zero the pad slots (piece 2 rows 226..255 per core) so the
                # matmuls never see NaN-pattern garbage
                for c in range(NCORES):
                    cb = (BASES[2] + c * PIECES[2][2]) // P
                    nc.vector.memset(
                        zsb[:, (cb + 1) * din:(cb + 2) * din], 0.0)
                for g, (off, valid, pad) in enumerate(PIECES):
                    base = BASES[g]
                    if l == 0:
                        for c in range(NCORES):
                            rows0 = c * NPC + off
                            cb = (base + c * pad) // P
                            kfull = valid // P
                            if kfull:
                                nc.scalar.dma_start(
                                    out=zsb[:, cb * din:(cb + kfull) * din]
                                        .rearrange("p (k f) -> p k f", f=din),
                                    in_=xin[rows0:rows0 + kfull * P, :]
                                        .rearrange("(k p) f -> p k f", p=P))
                            rem = valid - kfull * P
                            if rem:
                                nc.scalar.dma_start(
                                    out=zsb[:rem, (cb + kfull) * din:
                                            (cb + kfull + 1) * din],
                                    in_=xin[rows0 + kfull * P:rows0 + valid, :])
                    else:
                        t = zfp[l - 1][g]
                        if valid == pad:
                            k = NCORES * valid // P
                            cb = base // P
                            for k0 in range(0, k, 8):
                                k1 = min(k, k0 + 8)
                                nc.scalar.dma_start(
                                    out=zsb[:, (cb + k0) * din:(cb + k1) * din]
                                        .rearrange("p (k f) -> p k f", f=din),
                                    in_=t[k0 * P:k1 * P, :]
                                        .rearrange("(k p) f -> p k f", p=P))
                        else:
                            for c in range(NCORES):
                                rows0 = c * valid
                                cb = (base + c * pad) // P
                                kfull = valid // P
                                if kfull:
                                    nc.scalar.dma_start(
                                        out=zsb[:, cb * din:(cb + kfull) * din]
                                            .rearrange("p (k f) -> p k f", f=din),
                                        in_=t[rows0:rows0 + kfull * P, :]
                                            .rearrange("(k p) f -> p k f", p=P))
                                rem = valid - kfull * P
                                if rem:
                                    nc.scalar.dma_start(
                                        out=zsb[:rem, (cb + kfull) * din:
                                                (cb + kfull + 1) * din],
                                        in_=t[rows0 + kfull * P:rows0 + valid, :])

                # --- aggregation: h.T = z.T @ Aloc.T  (PSUM-accumulated)
                hps = [hpsum.tile([P, len(NG) * 512], f32,
                                  name=f"hps{mf}_{l}", tag=f"hps{mf}")
                       for mf in range(nkt)]

                def agg_mms(k, rhs_tile, rhs_off, first, final):
                    for mf in range(nkt):
                        for gi, (n0, nn) in enumerate(NG):
                            nc.tensor.matmul(
                                out=hps[mf][:, gi * 512: gi * 512 + nn],
                                lhsT=zsb[:, k * din + mf * P: k * din + (mf + 1) * P],
                                rhs=rhs_tile[:, rhs_off + n0: rhs_off + n0 + nn],
                                start=first,
                                stop=final,
                            )

                # Interleave SBUF-cached chunks between streamed batches so
                # TensorE never stalls on the A.T stream.
                cached = list(range(CA))
                head, rest = cached[:CFIRST], cached[CFIRST:]
                seq = [("C", k) for k in head]
                nb = max(1, NSB)
                per = [len(rest) * (bi + 1) // nb for bi in range(nb)]
                ci = 0
                for bi in range(NSB):
                    seq.append(("S", bi))
                    while ci < per[bi]:
                        seq.append(("C", rest[ci]))
                        ci += 1
                while ci < len(rest):
                    seq.append(("C", rest[ci]))
                    ci += 1

                nchunks = KC
                done = 0
                for kind, payload in seq:
                    if kind == "S":
                        b = payload
                        ks = [CA + b * ABATCH + j for j in range(ABATCH)
                              if CA + b * ABATCH + j < KC]
                        at_t = atp.tile([P, ABATCH * NSLOT], bf,
                                        name=f"at_{l}_{b}", tag="at")
                        if l == 0:
                            # layer 0 is DMA-bound: stream uint8, cast to
                            # bf16 on the otherwise-idle DVE/ACT engines
                            at8_t = a8p.tile([P, ABATCH * NSLOT],
                                             mybir.dt.uint8,
                                             name=f"at8_{l}_{b}", tag="at8")
                            nc.sync.dma_start(out=at8_t[:], in_=at8in[b, :, :])
                            if b < 3:
                                # ACT is busy with x-piece loads early on;
                                # cast the first batches entirely on DVE
                                nc.vector.tensor_copy(out=at_t[:], in_=at8_t[:])
                            else:
                                cut = ABATCH * NSLOT * 5 // 8
                                nc.vector.tensor_copy(
                                    out=at_t[:, :cut], in_=at8_t[:, :cut])
                                nc.scalar.activation(
                                    out=at_t[:, cut:], in_=at8_t[:, cut:],
                                    func=mybir.ActivationFunctionType.Copy)
                        else:
                            nc.sync.dma_start(out=at_t[:], in_=atsin[b, :, :])
                        for k in ks:
                            agg_mms(k, at_t, (k - CA - b * ABATCH) * NSLOT,
                                    done == 0, done == nchunks - 1)
                            done += 1
                    else:
                        k = payload
                        agg_mms(k, acache, k * NSLOT,
                                done == 0, done == nchunks - 1)
                        done += 1

                # --- h.T -> bf16 (plain, no hi/lo split)
                hhi = [hp.tile([P, MT * P], bf, name=f"hhi{mf}_{l}", tag=f"hhi{mf}")
                       for mf in range(nkt)]
                for mf in range(nkt):
                    for gi, (n0, nn) in enumerate(NG):
                        nc.vector.tensor_copy(
                            out=hhi[mf][:, n0:n0 + nn],
                            in_=hps[mf][:, gi * 512: gi * 512 + nn])

                # --- MLP over groups of 4 M-tiles (512-row free dim)
                zT = [zp.tile([P, MT * P], bf, name=f"zT{mo}_{l}",
                              tag=f"zT{mo}")
                      for mo in range(NKT2)]
                for gi, (g0, gm) in enumerate(GROUPS):
                    rows = gm * P
                    r0 = g0 * P
                    s1h = []
                    for mo in range(NKT2):
                        p1 = mlppool.tile([P, 512], f32,
                                          name=f"p1_{l}_{g0}_{mo}", tag="mlp")
                        for kt in range(nkt):
                            nc.tensor.matmul(
                                out=p1[:, :rows],
                                lhsT=wt[("w1h", l)][:, kt * DH + mo * P: kt * DH + (mo + 1) * P],
                                rhs=hhi[kt][:, r0:r0 + rows],
                                start=(kt == 0), stop=(kt == nkt - 1))
                        sh = sp.tile([P, 512], bf, name=f"s1h_{l}_{g0}_{mo}", tag=f"s1h{mo}")
                        nc.scalar.activation(
                            out=sh[:, :rows], in_=p1[:, :rows], func=RELU,
                            bias=wt[("b1", l)][:, mo:mo + 1])
                        s1h.append(sh)
                    for mo in range(NKT2):
                        p2 = mlppool.tile([P, 512], f32,
                                          name=f"p2_{l}_{g0}_{mo}", tag="mlp")
                        for kt in range(NKT2):
                            nc.tensor.matmul(
                                out=p2[:, :rows],
                                lhsT=wt[("w2h", l)][:, kt * DH + mo * P: kt * DH + (mo + 1) * P],
                                rhs=s1h[kt][:, :rows],
                                start=(kt == 0), stop=(kt == NKT2 - 1))
                        nc.scalar.activation(
                            out=zT[mo][:, r0:r0 + rows], in_=p2[:, :rows], func=RELU,
                            bias=wt[("b2", l)][:, mo:mo + 1])

                    # transpose this group's M-tiles back to row-major + store
                    for m in range(g0, g0 + gm):
                        rows_m = min(P, NPC - m * P)
                        tp = mlppool.tile([P, NKT2 * P], bf,
                                          name=f"tp_{l}_{m}", tag="mlp")
                        for mo in range(NKT2):
                            nc.tensor.transpose(
                                out=tp[:, mo * P:(mo + 1) * P],
                                in_=zT[mo][:, m * P:(m + 1) * P],
                                identity=identb_t[:])
                        zr = zrp.tile([P, NKT2 * P], bf,
                                      name=f"zr_{l}_{m}", tag="zr")
                        nc.vector.tensor_copy(out=zr[:], in_=tp[:])
                        dst = zout if last else zloc[l]
                        nc.sync.dma_start(
                            out=dst[m * P: m * P + rows_m, :],
                            in_=zr[:rows_m, :])

                    # halo exchange for this group's rows, overlapped with
                    # the rest of the MLP and the next layer's aggregation
                    if not last:
                        off, valid, pad = PIECES[gi]
                        nc.gpsimd.collective_compute(
                            "AllGather",
                            mybir.AluOpType.bypass,
                            replica_groups=[list(range(NCORES))],
                            ins=[zloc[l][off:off + valid, :].opt()],
                            outs=[zfp[l][gi][:, :].opt()],
                        )

    # populates extended-inst ISA bytes + inserts GPSIMD library loads
    nc.compile()
    return nc


# --------------------------------------------------------------------------
# entry point
# --------------------------------------------------------------------------

def _make_in_maps(inputs, cfg, aca, ats, at8s):
    DH, L = cfg["DH"], cfg["L"]
    x_bf = np.ascontiguousarray(np.asarray(inputs["x"], dtype=np.float32)).astype(BF16)
    identb = np.eye(P, dtype=np.float32).astype(BF16)

    shared = {"x_bf": x_bf, "identb": identb,
              "dummyin": np.zeros((8, 2), BF16)}
    for l in range(L):
        w1 = np.asarray(inputs[f"w1_{l}"], dtype=np.float32)
        w2 = np.asarray(inputs[f"w2_{l}"], dtype=np.float32)
        shared[f"w1h_{l}"] = w1.astype(BF16)
        shared[f"w2h_{l}"] = w2.astype(BF16)
        shared[f"b1_{l}"] = np.asarray(
            inputs[f"b1_{l}"], dtype=np.float32).reshape(DH, 1)
        shared[f"b2_{l}"] = np.asarray(
            inputs[f"b2_{l}"], dtype=np.float32).reshape(DH, 1)

    in_maps = []
    for c in range(NCORES):
        m = dict(shared)
        m["aca"] = aca[c]
        m["ats"] = ats[c]
        m["at8s"] = at8s[c]
        in_maps.append(m)
    return in_maps


def get_program(inputs):
    """Build (or fetch cached) the bass program + per-core input maps."""
    cfg = _config(inputs)
    aca, ats, at8s = _prep_at(inputs["edge_index"], cfg["N"])
    key = (cfg["N"], cfg["DIN"], cfg["DH"], cfg["L"])
    if key not in _BUILD_CACHE:
        _BUILD_CACHE[key] = _build(cfg["N"], cfg["DIN"], cfg["DH"], cfg["L"])
    nc = _BUILD_CACHE[key]
    in_maps = _make_in_maps(inputs, cfg, aca, ats, at8s)
    return nc, in_maps, cfg


def kernel(**inputs):
    nc, in_maps, cfg = get_program(inputs)

    if os.environ.get("KERNEL_USE_SIM"):
        from concourse.bass_interp import MultiCoreSim
        sim = MultiCoreSim(nc, num_cores=NCORES)
        cores = list(sim.cores.values())
        for cid, cs in enumerate(cores):
            for name, val in in_maps[cid].items():
                cs.tensor(name)[:] = val
        sim.simulate(check_with_hw=False)
        parts = [np.asarray(cs.tensor("zout")) for cs in cores]
    else:
        from concourse import bass_utils
        res = bass_utils.run_bass_kernel_spmd(
            nc, in_maps, core_ids=list(range(NCORES)),
            trace=bool(os.environ.get("KERNEL_TRACE")),
        )
        kernel.last_results = res
        parts = [res.results[c]["zout"] for c in range(NCORES)]

    out = np.concatenate(parts, axis=0).astype(np.float32)
    return out
